# revision 35
# baseline (speedup 1.0000x reference)
"""BitLinear158 Trainium2 kernel (per-core body + host driver).

v7: no activation quantization (the reference's per-token int8 scale
cancels algebraically, y ~= x @ w.T within its own quant noise ~0.9%),
plus fp8 DoubleRow matmul on the first 4 of 16 k-chunks. The fp8 part
casts x -> e4m3 on-device (DVE) and runs 256-deep contractions at the
same per-instruction cost as 128-deep bf16 (measured), cutting PE time
12.5%. Total rel err ~1.6e-2 vs the 2e-2 gate (deterministic).

Per core: x_shard [M_LOC, K] bf16 -> y [M_LOC, N] bf16 against
host-unpacked ternary wT [K, N] (bf16 for kc>=4, e4m3 pairs for kc<4).

Queue discipline (HWDGE completion sems assume in-order completion per
queue; xbar transposes complete out of order w.r.t. direct DMAs):
  sync   : DMA transposes only (bf16 xT tiles + uint16 views of x8 pairs)
  scalar : w slices + y stores (direct DMAs only)
  gpsimd : SWDGE - w slices + x slice loads / x8 stores for the fp8 path
  DVE    : bf16->e4m3 casts + PSUM->bf16 drains
"""

import sys

sys.path.insert(0, "/opt/trn_rl_repo")

from contextlib import ExitStack

import numpy as np
import ml_dtypes

import concourse.bass as bass
import concourse.tile as tile
from concourse import bacc, mybir
from concourse import bass_utils

P = 128
M_LOC = 4096      # tokens per core
K = 2048          # in features
N = 2048          # out features
KC = K // P       # 16 k-chunks
KF8 = 4           # k-chunks 0..3 run in fp8 DoubleRow
JF8 = KF8 // 2    # 2 DoubleRow pair-blocks
NT = M_LOC // P   # 32 m-tiles per core
CHUNK_MTS = [2, 2, 4, 4, 4, 4, 4, 4, 2, 2]
assert sum(CHUNK_MTS) == NT
CHUNK_STARTS = [sum(CHUNK_MTS[:i]) for i in range(len(CHUNK_MTS))]
CHUNKS = len(CHUNK_MTS)
MAX_CHUNK_MT = max(CHUNK_MTS)
N_TILE = 512
NTN = N // N_TILE                  # 4
N_CORES = 8

BF16 = mybir.dt.bfloat16
F32 = mybir.dt.float32
F8 = mybir.dt.float8e4
U16 = mybir.dt.uint16


def build_kernel(replays: int = 1):
    nc = bacc.Bacc("TRN2", target_bir_lowering=False, debug=False, num_devices=N_CORES)
    x = nc.dram_tensor("x", [M_LOC, K], BF16, kind="ExternalInput").ap()
    wT = nc.dram_tensor("wT", [K, N], BF16, kind="ExternalInput").ap()
    w8 = nc.dram_tensor("w8", [JF8, P, 2, N], F8, kind="ExternalInput").ap()
    y = nc.dram_tensor("y", [M_LOC, N], BF16, kind="ExternalOutput").ap()

    y_tiled = y.rearrange("(t p) n -> t p n", p=P)
    # bf16 w pair-blocks j cover kc {2j, 2j+1}; only j >= JF8 are used.
    wT_pair = wT.rearrange("(j two p) n -> j p two n", two=2, p=P)

    with tile.TileContext(nc) as tc, ExitStack() as ctx:
        wbuf = ctx.enter_context(tc.tile_pool(name="wbuf", bufs=1))
        xT_pool = ctx.enter_context(tc.tile_pool(name="xT", bufs=4))
        x8T_pool = ctx.enter_context(tc.tile_pool(name="x8T", bufs=4))
        x8in = ctx.enter_context(tc.tile_pool(name="x8in", bufs=4))
        x8q = ctx.enter_context(tc.tile_pool(name="x8q", bufs=4))
        yout = ctx.enter_context(tc.tile_pool(name="yout", bufs=8))
        psum = ctx.enter_context(tc.tile_pool(name="psum", bufs=8, space="PSUM"))
        dram = ctx.enter_context(tc.tile_pool(name="dram", bufs=1, space="DRAM"))

        x8_dram = (
            dram.tile([M_LOC, KF8 * P], F8, tag="x8d", name="x8d") if KF8 else None
        )
        # u16 col c packs fp8 (k=2c, 2c+1): transposing it gives k-pair
        # interleaved tiles which DVE deinterleaves into [p, 2, m] blocks.
        x8_u16 = x8_dram[:].bitcast(U16) if KF8 else None

        # x[:, 0:512] -> e4m3 staging via SWDGE cast-DMAs (DRAM->DRAM with
        # on-the-fly bf16->fp8 cast) on the otherwise-idle gpsimd queue.
        # A small first block unblocks chunks 0-1 fast; the rest stream in
        # well ahead of their consuming chunks.
        PREP_SPANS = [(0, 512), (512, 1280), (1280, 2304), (2304, 3328), (3328, 4096)]

        def prep_block(b):
            r0, r1 = PREP_SPANS[b]
            nc.gpsimd.dma_start(
                x8_dram[r0:r1, :], x[r0:r1, 0 : KF8 * P]
            )

        if KF8:
            for b in range(len(PREP_SPANS)):
                prep_block(b)

        # w on the scalar HWDGE queue in PE consumption order: bf16 pair
        # slices (kc 4..15) first, fp8 pairs last (consumed at the tail of
        # each accumulation group).
        wt = {}
        for j in range(JF8, KC // 2):
            wt[j] = wbuf.tile([P, 2, N], BF16, tag=f"w{j}", name=f"w{j}")
            nc.scalar.dma_start(wt[j][:], wT_pair[j])
        w8t = [wbuf.tile([P, 2, N], F8, tag=f"w8_{j}", name=f"w8_{j}") for j in range(JF8)]
        for j in range(JF8):
            nc.scalar.dma_start(w8t[j][:], w8[j])

        def w_ap(kc, nt):
            return wt[kc // 2][:, kc % 2, nt * N_TILE : (nt + 1) * N_TILE]

        for rep in range(replays):

            def transpose_chunk(c):
                cm = CHUNK_MTS[c]
                m0 = CHUNK_STARTS[c] * P
                tiles = {}
                for kc in range(KF8, KC):
                    tt = xT_pool.tile(
                        [P, MAX_CHUNK_MT * P], BF16, tag=f"xT{kc}", name=f"xT{kc}"
                    )
                    nc.sync.dma_start_transpose(
                        tt[:, : cm * P],
                        x[m0 : m0 + cm * P, kc * P : (kc + 1) * P],
                    )
                    tiles[kc] = tt
                for j in range(JF8):
                    traw = x8T_pool.tile(
                        [P, MAX_CHUNK_MT * P], U16, tag=f"x8R{j}", name=f"x8R{j}"
                    )
                    nc.sync.dma_start_transpose(
                        traw[:, : cm * P],
                        x8_u16[m0 : m0 + cm * P, j * P : (j + 1) * P],
                    )
                    # deinterleave (p, 2m+i) -> [p, i, m] blocks on DVE
                    ft = x8T_pool.tile(
                        [P, 2, MAX_CHUNK_MT * P], F8, tag=f"x8T{j}", name=f"x8T{j}"
                    )
                    tview = traw[:, : cm * P].bitcast(F8).rearrange(
                        "p (m two) -> p two m", two=2
                    )
                    for i in range(2):
                        nc.vector.tensor_copy(ft[:, i, : cm * P], tview[:, i, :])
                    tiles[f"f8_{j}"] = ft
                return tiles

            def emit_matmuls(ps, xT, mi, nt):
                for kc in range(KF8, KC):
                    nc.tensor.matmul(
                        ps[:],
                        xT[kc][:, mi * P : (mi + 1) * P],
                        w_ap(kc, nt),
                        start=(kc == KF8),
                        stop=(JF8 == 0 and kc == KC - 1),
                    )
                for j in range(JF8):
                    nc.tensor.matmul(
                        ps[:],
                        xT[f"f8_{j}"][:, :, mi * P : (mi + 1) * P],
                        w8t[j][:, :, nt * N_TILE : (nt + 1) * N_TILE],
                        start=False,
                        stop=(j == JF8 - 1),
                        perf_mode=mybir.MatmulPerfMode.DoubleRow,
                    )

            def matmul_mtile(c, mi, xT):
                mt = CHUNK_STARTS[c] + mi
                y_sb = yout.tile([P, N], BF16, tag="y_sb", name="y_sb")
                for nt in range(NTN):
                    ps = psum.tile([P, N_TILE], F32, tag="ps", name="ps")
                    emit_matmuls(ps, xT, mi, nt)
                    nc.vector.tensor_copy(
                        y_sb[:, nt * N_TILE : (nt + 1) * N_TILE], ps[:]
                    )
                nc.scalar.dma_start(y_tiled[mt], y_sb[:])

            def matmul_chunk_kc_outer(c, xT):
                # kc-outer over the whole (small) chunk: the PE consumes each
                # w k-slice as it lands instead of stalling the first m-tile
                # on the full w load. Needs cm*NTN <= 8 PSUM banks.
                cm = CHUNK_MTS[c]
                assert cm * NTN <= 8
                pss = [
                    [psum.tile([P, N_TILE], F32, tag="ps", name="ps") for _ in range(NTN)]
                    for _ in range(cm)
                ]
                for kc in range(KF8, KC):
                    for mi in range(cm):
                        for nt in range(NTN):
                            nc.tensor.matmul(
                                pss[mi][nt][:],
                                xT[kc][:, mi * P : (mi + 1) * P],
                                w_ap(kc, nt),
                                start=(kc == KF8),
                                stop=(JF8 == 0 and kc == KC - 1),
                            )
                for j in range(JF8):
                    for mi in range(cm):
                        for nt in range(NTN):
                            nc.tensor.matmul(
                                pss[mi][nt][:],
                                xT[f"f8_{j}"][:, :, mi * P : (mi + 1) * P],
                                w8t[j][:, :, nt * N_TILE : (nt + 1) * N_TILE],
                                start=False,
                                stop=(j == JF8 - 1),
                                perf_mode=mybir.MatmulPerfMode.DoubleRow,
                            )
                for mi in range(cm):
                    mt = CHUNK_STARTS[c] + mi
                    y_sb = yout.tile([P, N], BF16, tag="y_sb", name="y_sb")
                    for nt in range(NTN):
                        nc.vector.tensor_copy(
                            y_sb[:, nt * N_TILE : (nt + 1) * N_TILE], pss[mi][nt][:]
                        )
                    nc.scalar.dma_start(y_tiled[mt], y_sb[:])

            xT_map = {0: transpose_chunk(0)}
            for c in range(CHUNKS):
                if c + 1 < CHUNKS:
                    xT_map[c + 1] = transpose_chunk(c + 1)
                if c <= 1:
                    matmul_chunk_kc_outer(c, xT_map[c])
                else:
                    for mi in range(CHUNK_MTS[c]):
                        matmul_mtile(c, mi, xT_map[c])
                del xT_map[c]

    nc.compile()
    return nc


def unpack_w(packed_weight: np.ndarray, weight_scale: np.ndarray):
    planes = [((packed_weight >> (2 * i)) & 3) for i in range(4)]
    w = np.concatenate(planes, axis=0).astype(np.float32) - 1.0  # [N, K]
    ws = np.float32(weight_scale.reshape(-1)[0])
    wTf = np.ascontiguousarray((w / ws).T)  # [K, N] f32
    wT = wTf.astype(ml_dtypes.bfloat16)
    # fp8 pair planes: w8[j, p, i, n] = wT[256j + 2p + i, n]
    w8 = np.ascontiguousarray(
        wTf[: KF8 * P].reshape(JF8, P, 2, N)
    ).astype(ml_dtypes.float8_e4m3fn)
    return wT, w8


_CACHE = {}


def run(x: np.ndarray, packed_weight: np.ndarray, weight_scale: np.ndarray,
        trace: bool = False, replays: int = 1, tmpdir=None):
    """x: [B, S, K] bf16 -> y [B, S, N] bf16 (full, unsharded)."""
    key = (replays,)
    if key not in _CACHE:
        _CACHE[key] = build_kernel(replays)
    nc = _CACHE[key]

    B, S, D = x.shape
    M = B * S
    assert M == M_LOC * N_CORES and D == K
    wT, w8 = unpack_w(packed_weight, weight_scale)
    shards = np.ascontiguousarray(np.asarray(x).reshape(N_CORES, M_LOC, K))
    in_maps = [{"x": shards[i], "wT": wT, "w8": w8} for i in range(N_CORES)]
    res = bass_utils.run_bass_kernel_spmd(
        nc, in_maps, core_ids=list(range(N_CORES)), trace=trace, tmpdir=tmpdir
    )
    y = np.stack([res.results[i]["y"] for i in range(N_CORES)], axis=0)
    return y.reshape(B, S, N), res


def kernel(x, packed_weight, weight_scale):
    """Harness entrypoint: FULL inputs -> FULL output.

    x: [4, 8192, 2048] bf16; packed_weight: [512, 2048] uint8;
    weight_scale: [1] bf16.  Returns [4, 8192, 2048] bf16.
    Sharding: data-parallel over tokens across the 8 NeuronCores;
    the (host-unpacked) ternary weight is replicated.
    """
    x = np.asarray(x)
    packed_weight = np.asarray(packed_weight)
    weight_scale = np.asarray(weight_scale)
    y, _ = run(x, packed_weight, weight_scale)
    return y


# revision 38
# speedup vs baseline: 1.1783x; 1.1783x over previous
"""BitLinear158 Trainium2 kernel (per-core body + host driver).

v7: no activation quantization (the reference's per-token int8 scale
cancels algebraically, y ~= x @ w.T within its own quant noise ~0.9%),
plus fp8 DoubleRow matmul on the first 4 of 16 k-chunks. The fp8 part
casts x -> e4m3 on-device (DVE) and runs 256-deep contractions at the
same per-instruction cost as 128-deep bf16 (measured), cutting PE time
12.5%. Total rel err ~1.6e-2 vs the 2e-2 gate (deterministic).

Per core: x_shard [M_LOC, K] bf16 -> y [M_LOC, N] bf16 against
host-unpacked ternary wT [K, N] (bf16 for kc>=4, e4m3 pairs for kc<4).

Queue discipline (HWDGE completion sems assume in-order completion per
queue; xbar transposes complete out of order w.r.t. direct DMAs):
  sync   : DMA transposes only (bf16 xT tiles + uint16 views of x8 pairs)
  scalar : w slices + y stores (direct DMAs only)
  gpsimd : SWDGE - w slices + x slice loads / x8 stores for the fp8 path
  DVE    : bf16->e4m3 casts + PSUM->bf16 drains
"""

import sys

sys.path.insert(0, "/opt/trn_rl_repo")

from contextlib import ExitStack

import numpy as np
import ml_dtypes

import concourse.bass as bass
import concourse.tile as tile
from concourse import bacc, mybir
from concourse import bass_utils

P = 128
M_LOC = 4096      # tokens per core
K = 2048          # in features
N = 2048          # out features
KC = K // P       # 16 k-chunks
KF8 = 4           # k-chunks 0..3 run in fp8 DoubleRow
JF8 = KF8 // 2    # 2 DoubleRow pair-blocks
NT = M_LOC // P   # 32 m-tiles per core
CHUNK_MTS = [2, 2, 4, 4, 4, 4, 4, 4, 2, 2]
assert sum(CHUNK_MTS) == NT
CHUNK_STARTS = [sum(CHUNK_MTS[:i]) for i in range(len(CHUNK_MTS))]
CHUNKS = len(CHUNK_MTS)
MAX_CHUNK_MT = max(CHUNK_MTS)
N_TILE = 512
NTN = N // N_TILE                  # 4
N_CORES = 8

BF16 = mybir.dt.bfloat16
F32 = mybir.dt.float32
F8 = mybir.dt.float8e4
U16 = mybir.dt.uint16


def build_kernel(replays: int = 1):
    nc = bacc.Bacc("TRN2", target_bir_lowering=False, debug=False, num_devices=N_CORES)
    x = nc.dram_tensor("x", [M_LOC, K], BF16, kind="ExternalInput").ap()
    wT = nc.dram_tensor("wT", [K, N], BF16, kind="ExternalInput").ap()
    w8 = nc.dram_tensor("w8", [JF8, P, 2, N], F8, kind="ExternalInput").ap()
    y = nc.dram_tensor("y", [M_LOC, N], BF16, kind="ExternalOutput").ap()

    y_tiled = y.rearrange("(t p) n -> t p n", p=P)
    # bf16 w pair-blocks j cover kc {2j, 2j+1}; only j >= JF8 are used.
    wT_pair = wT.rearrange("(j two p) n -> j p two n", two=2, p=P)

    with tile.TileContext(nc) as tc, ExitStack() as ctx:
        wbuf = ctx.enter_context(tc.tile_pool(name="wbuf", bufs=1))
        xT_pool = ctx.enter_context(tc.tile_pool(name="xT", bufs=4))
        x8T_pool = ctx.enter_context(tc.tile_pool(name="x8T", bufs=4))
        x8in = ctx.enter_context(tc.tile_pool(name="x8in", bufs=4))
        x8q = ctx.enter_context(tc.tile_pool(name="x8q", bufs=4))
        yout = ctx.enter_context(tc.tile_pool(name="yout", bufs=8))
        psum = ctx.enter_context(tc.tile_pool(name="psum", bufs=8, space="PSUM"))
        dram = ctx.enter_context(tc.tile_pool(name="dram", bufs=1, space="DRAM"))

        # w in PE consumption order, alternating the scalar HWDGE and
        # gpsimd SWDGE queues: bf16 pair slices (kc 4..15) first, fp8
        # pairs last (consumed at the tail of each accumulation group).
        wt = {}
        w8t = [wbuf.tile([P, 2, N], F8, tag=f"w8_{j}", name=f"w8_{j}") for j in range(JF8)]
        ch = [nc.scalar, nc.gpsimd]
        for t, j in enumerate(range(JF8, KC // 2)):
            wt[j] = wbuf.tile([P, 2, N], BF16, tag=f"w{j}", name=f"w{j}")
            ch[t % 2].dma_start(wt[j][:], wT_pair[j])
        for j in range(JF8):
            ch[j % 2].dma_start(w8t[j][:], w8[j])

        def w_ap(kc, nt):
            return wt[kc // 2][:, kc % 2, nt * N_TILE : (nt + 1) * N_TILE]

        for rep in range(replays):

            def transpose_chunk(c):
                cm = CHUNK_MTS[c]
                m0 = CHUNK_STARTS[c] * P
                tiles = {}
                for kc in range(KC):
                    tt = xT_pool.tile(
                        [P, MAX_CHUNK_MT * P], BF16, tag=f"xT{kc}", name=f"xT{kc}"
                    )
                    nc.sync.dma_start_transpose(
                        tt[:, : cm * P],
                        x[m0 : m0 + cm * P, kc * P : (kc + 1) * P],
                    )
                    tiles[kc] = tt
                # e4m3 conversion after the transpose, entirely in SBUF:
                # DVE casts the transposed bf16 kc<KF8 tiles into [p, 2, m]
                # pair blocks for the DoubleRow matmuls.
                for j in range(JF8):
                    ft = x8T_pool.tile(
                        [P, 2, MAX_CHUNK_MT * P], F8, tag=f"x8T{j}", name=f"x8T{j}"
                    )
                    for i in range(2):
                        nc.vector.tensor_copy(
                            ft[:, i, : cm * P], tiles[2 * j + i][:, : cm * P]
                        )
                    tiles[f"f8_{j}"] = ft
                return tiles

            def emit_matmuls(ps, xT, mi, nt):
                for kc in range(KF8, KC):
                    nc.tensor.matmul(
                        ps[:],
                        xT[kc][:, mi * P : (mi + 1) * P],
                        w_ap(kc, nt),
                        start=(kc == KF8),
                        stop=(JF8 == 0 and kc == KC - 1),
                    )
                for j in range(JF8):
                    nc.tensor.matmul(
                        ps[:],
                        xT[f"f8_{j}"][:, :, mi * P : (mi + 1) * P],
                        w8t[j][:, :, nt * N_TILE : (nt + 1) * N_TILE],
                        start=False,
                        stop=(j == JF8 - 1),
                        perf_mode=mybir.MatmulPerfMode.DoubleRow,
                    )

            def matmul_mtile(c, mi, xT):
                mt = CHUNK_STARTS[c] + mi
                y_sb = yout.tile([P, N], BF16, tag="y_sb", name="y_sb")
                for nt in range(NTN):
                    ps = psum.tile([P, N_TILE], F32, tag="ps", name="ps")
                    emit_matmuls(ps, xT, mi, nt)
                    nc.vector.tensor_copy(
                        y_sb[:, nt * N_TILE : (nt + 1) * N_TILE], ps[:]
                    )
                nc.scalar.dma_start(y_tiled[mt], y_sb[:])

            def matmul_chunk_kc_outer(c, xT):
                # kc-outer over the whole (small) chunk: the PE consumes each
                # w k-slice as it lands instead of stalling the first m-tile
                # on the full w load. Needs cm*NTN <= 8 PSUM banks.
                cm = CHUNK_MTS[c]
                assert cm * NTN <= 8
                pss = [
                    [psum.tile([P, N_TILE], F32, tag="ps", name="ps") for _ in range(NTN)]
                    for _ in range(cm)
                ]
                for kc in range(KF8, KC):
                    for mi in range(cm):
                        for nt in range(NTN):
                            nc.tensor.matmul(
                                pss[mi][nt][:],
                                xT[kc][:, mi * P : (mi + 1) * P],
                                w_ap(kc, nt),
                                start=(kc == KF8),
                                stop=(JF8 == 0 and kc == KC - 1),
                            )
                for j in range(JF8):
                    for mi in range(cm):
                        for nt in range(NTN):
                            nc.tensor.matmul(
                                pss[mi][nt][:],
                                xT[f"f8_{j}"][:, :, mi * P : (mi + 1) * P],
                                w8t[j][:, :, nt * N_TILE : (nt + 1) * N_TILE],
                                start=False,
                                stop=(j == JF8 - 1),
                                perf_mode=mybir.MatmulPerfMode.DoubleRow,
                            )
                for mi in range(cm):
                    mt = CHUNK_STARTS[c] + mi
                    y_sb = yout.tile([P, N], BF16, tag="y_sb", name="y_sb")
                    for nt in range(NTN):
                        nc.vector.tensor_copy(
                            y_sb[:, nt * N_TILE : (nt + 1) * N_TILE], pss[mi][nt][:]
                        )
                    nc.scalar.dma_start(y_tiled[mt], y_sb[:])

            xT_map = {0: transpose_chunk(0)}
            for c in range(CHUNKS):
                if c + 1 < CHUNKS:
                    xT_map[c + 1] = transpose_chunk(c + 1)
                if c <= 1:
                    matmul_chunk_kc_outer(c, xT_map[c])
                else:
                    for mi in range(CHUNK_MTS[c]):
                        matmul_mtile(c, mi, xT_map[c])
                del xT_map[c]

    nc.compile()
    return nc


def unpack_w(packed_weight: np.ndarray, weight_scale: np.ndarray):
    planes = [((packed_weight >> (2 * i)) & 3) for i in range(4)]
    w = np.concatenate(planes, axis=0).astype(np.float32) - 1.0  # [N, K]
    ws = np.float32(weight_scale.reshape(-1)[0])
    wTf = np.ascontiguousarray((w / ws).T)  # [K, N] f32
    wT = wTf.astype(ml_dtypes.bfloat16)
    # fp8 pair planes: w8[j, p, i, n] = wT[128*(2j+i) + p, n]
    w8 = np.ascontiguousarray(
        wTf[: KF8 * P].reshape(JF8, 2, P, N).transpose(0, 2, 1, 3)
    ).astype(ml_dtypes.float8_e4m3fn)
    return wT, w8


_CACHE = {}


def run(x: np.ndarray, packed_weight: np.ndarray, weight_scale: np.ndarray,
        trace: bool = False, replays: int = 1, tmpdir=None):
    """x: [B, S, K] bf16 -> y [B, S, N] bf16 (full, unsharded)."""
    key = (replays,)
    if key not in _CACHE:
        _CACHE[key] = build_kernel(replays)
    nc = _CACHE[key]

    B, S, D = x.shape
    M = B * S
    assert M == M_LOC * N_CORES and D == K
    wT, w8 = unpack_w(packed_weight, weight_scale)
    shards = np.ascontiguousarray(np.asarray(x).reshape(N_CORES, M_LOC, K))
    in_maps = [{"x": shards[i], "wT": wT, "w8": w8} for i in range(N_CORES)]
    res = bass_utils.run_bass_kernel_spmd(
        nc, in_maps, core_ids=list(range(N_CORES)), trace=trace, tmpdir=tmpdir
    )
    y = np.stack([res.results[i]["y"] for i in range(N_CORES)], axis=0)
    return y.reshape(B, S, N), res


def kernel(x, packed_weight, weight_scale):
    """Harness entrypoint: FULL inputs -> FULL output.

    x: [4, 8192, 2048] bf16; packed_weight: [512, 2048] uint8;
    weight_scale: [1] bf16.  Returns [4, 8192, 2048] bf16.
    Sharding: data-parallel over tokens across the 8 NeuronCores;
    the (host-unpacked) ternary weight is replicated.
    """
    x = np.asarray(x)
    packed_weight = np.asarray(packed_weight)
    weight_scale = np.asarray(weight_scale)
    y, _ = run(x, packed_weight, weight_scale)
    return y


# revision 42
# speedup vs baseline: 1.2889x; 1.0939x over previous
"""BitLinear158 Trainium2 kernel (per-core body + host driver).

v7: no activation quantization (the reference's per-token int8 scale
cancels algebraically, y ~= x @ w.T within its own quant noise ~0.9%),
plus fp8 DoubleRow matmul on the first 4 of 16 k-chunks. The fp8 part
casts x -> e4m3 on-device (DVE) and runs 256-deep contractions at the
same per-instruction cost as 128-deep bf16 (measured), cutting PE time
12.5%. Total rel err ~1.6e-2 vs the 2e-2 gate (deterministic).

Per core: x_shard [M_LOC, K] bf16 -> y [M_LOC, N] bf16 against
host-unpacked ternary wT [K, N] (bf16 for kc>=4, e4m3 pairs for kc<4).

Queue discipline (HWDGE completion sems assume in-order completion per
queue; xbar transposes complete out of order w.r.t. direct DMAs):
  sync   : DMA transposes only (bf16 xT tiles + uint16 views of x8 pairs)
  scalar : w slices + y stores (direct DMAs only)
  gpsimd : SWDGE - w slices + x slice loads / x8 stores for the fp8 path
  DVE    : bf16->e4m3 casts + PSUM->bf16 drains
"""

import sys

sys.path.insert(0, "/opt/trn_rl_repo")

from contextlib import ExitStack

import numpy as np
import ml_dtypes

import concourse.bass as bass
import concourse.tile as tile
from concourse import bacc, mybir
from concourse import bass_utils

P = 128
M_LOC = 4096      # tokens per core
K = 2048          # in features
N = 2048          # out features
KC = K // P       # 16 k-chunks
KF8 = 4           # k-chunks 0..3 run in fp8 DoubleRow
JF8 = KF8 // 2    # 2 DoubleRow pair-blocks
NT = M_LOC // P   # 32 m-tiles per core
CHUNK_MTS = [2, 2, 4, 4, 4, 4, 4, 4, 2, 2]
assert sum(CHUNK_MTS) == NT
CHUNK_STARTS = [sum(CHUNK_MTS[:i]) for i in range(len(CHUNK_MTS))]
CHUNKS = len(CHUNK_MTS)
MAX_CHUNK_MT = max(CHUNK_MTS)
N_TILE = 512
NTN = N // N_TILE                  # 4
N_CORES = 8

BF16 = mybir.dt.bfloat16
F32 = mybir.dt.float32
F8 = mybir.dt.float8e4
U16 = mybir.dt.uint16


def build_kernel(replays: int = 1):
    nc = bacc.Bacc("TRN2", target_bir_lowering=False, debug=False, num_devices=N_CORES)
    x = nc.dram_tensor("x", [M_LOC, K], BF16, kind="ExternalInput").ap()
    wT = nc.dram_tensor("wT", [K, N], BF16, kind="ExternalInput").ap()
    w8 = nc.dram_tensor("w8", [JF8, P, 2, N], F8, kind="ExternalInput").ap()
    y = nc.dram_tensor("y", [M_LOC, N], BF16, kind="ExternalOutput").ap()

    y_tiled = y.rearrange("(t p) n -> t p n", p=P)
    # bf16 w pair-blocks j cover kc {2j, 2j+1}; only j >= JF8 are used.
    wT_pair = wT.rearrange("(j two p) n -> j p two n", two=2, p=P)

    with tile.TileContext(nc) as tc, ExitStack() as ctx:
        wbuf = ctx.enter_context(tc.tile_pool(name="wbuf", bufs=1))
        xT_pool = ctx.enter_context(tc.tile_pool(name="xT", bufs=2))
        x8T_pool = ctx.enter_context(tc.tile_pool(name="x8T", bufs=2))
        x8in = ctx.enter_context(tc.tile_pool(name="x8in", bufs=4))
        x8q = ctx.enter_context(tc.tile_pool(name="x8q", bufs=4))
        yout = ctx.enter_context(tc.tile_pool(name="yout", bufs=8))
        psum = ctx.enter_context(tc.tile_pool(name="psum", bufs=8, space="PSUM"))
        dram = ctx.enter_context(tc.tile_pool(name="dram", bufs=1, space="DRAM"))

        # w in PE consumption order, alternating the scalar HWDGE and
        # gpsimd SWDGE queues: bf16 pair slices (kc 4..15) first, fp8
        # pairs last (consumed at the tail of each accumulation group).
        wt = {}
        w8t = [wbuf.tile([P, 2, N], F8, tag=f"w8_{j}", name=f"w8_{j}") for j in range(JF8)]
        ch = [nc.scalar, nc.gpsimd]
        for t, j in enumerate(range(JF8, KC // 2)):
            wt[j] = wbuf.tile([P, 2, N], BF16, tag=f"w{j}", name=f"w{j}")
            ch[t % 2].dma_start(wt[j][:], wT_pair[j])
        for j in range(JF8):
            ch[j % 2].dma_start(w8t[j][:], w8[j])

        def w_ap(kc, nt):
            return wt[kc // 2][:, kc % 2, nt * N_TILE : (nt + 1) * N_TILE]

        for rep in range(replays):
            # Transposes come in units of U_MT m-tiles, decoupled from the
            # (smaller) compute chunks: DMA_TRANSPOSE has a fixed ~1.26us
            # issue cost on the sync queue regardless of size, so fewer,
            # bigger transposes keep the queue far ahead of the PE.
            U_MT = 8
            UNITS = NT // U_MT

            def transpose_unit(u):
                m0 = u * U_MT * P
                tiles = {}
                for kc in range(KC):
                    tt = xT_pool.tile(
                        [P, U_MT * P], BF16, tag=f"xT{kc}", name=f"xT{kc}"
                    )
                    nc.sync.dma_start_transpose(
                        tt[:], x[m0 : m0 + U_MT * P, kc * P : (kc + 1) * P]
                    )
                    tiles[kc] = tt
                # e4m3 conversion after the transpose, entirely in SBUF:
                # DVE casts the transposed bf16 kc<KF8 tiles into [p, 2, m]
                # pair blocks for the DoubleRow matmuls.
                for j in range(JF8):
                    ft = x8T_pool.tile(
                        [P, 2, U_MT * P], F8, tag=f"x8T{j}", name=f"x8T{j}"
                    )
                    for i in range(2):
                        nc.vector.tensor_copy(ft[:, i, :], tiles[2 * j + i][:])
                    tiles[f"f8_{j}"] = ft
                return tiles

            def emit_matmuls(ps, xT, off, nt):
                for kc in range(KF8, KC):
                    nc.tensor.matmul(
                        ps[:],
                        xT[kc][:, off : off + P],
                        w_ap(kc, nt),
                        start=(kc == KF8),
                        stop=(JF8 == 0 and kc == KC - 1),
                    )
                for j in range(JF8):
                    nc.tensor.matmul(
                        ps[:],
                        xT[f"f8_{j}"][:, :, off : off + P],
                        w8t[j][:, :, nt * N_TILE : (nt + 1) * N_TILE],
                        start=False,
                        stop=(j == JF8 - 1),
                        perf_mode=mybir.MatmulPerfMode.DoubleRow,
                    )

            def matmul_mtile(c, mi, units):
                mt = CHUNK_STARTS[c] + mi
                xT = units[mt // U_MT]
                off = (mt % U_MT) * P
                y_sb = yout.tile([P, N], BF16, tag="y_sb", name="y_sb")
                for nt in range(NTN):
                    ps = psum.tile([P, N_TILE], F32, tag="ps", name="ps")
                    emit_matmuls(ps, xT, off, nt)
                    nc.vector.tensor_copy(
                        y_sb[:, nt * N_TILE : (nt + 1) * N_TILE], ps[:]
                    )
                nc.scalar.dma_start(y_tiled[mt], y_sb[:])

            def matmul_chunk_kc_outer(c, units):
                # kc-outer over the whole (small) chunk: the PE consumes each
                # w k-slice as it lands instead of stalling the first m-tile
                # on the full w load. Needs cm*NTN <= 8 PSUM banks.
                xT = units[CHUNK_STARTS[c] // U_MT]
                cm = CHUNK_MTS[c]
                assert cm * NTN <= 8
                pss = [
                    [psum.tile([P, N_TILE], F32, tag="ps", name="ps") for _ in range(NTN)]
                    for _ in range(cm)
                ]
                for kc in range(KF8, KC):
                    for mi in range(cm):
                        off = (CHUNK_STARTS[c] + mi) % U_MT * P
                        for nt in range(NTN):
                            nc.tensor.matmul(
                                pss[mi][nt][:],
                                xT[kc][:, off : off + P],
                                w_ap(kc, nt),
                                start=(kc == KF8),
                                stop=(JF8 == 0 and kc == KC - 1),
                            )
                for j in range(JF8):
                    for mi in range(cm):
                        off = (CHUNK_STARTS[c] + mi) % U_MT * P
                        for nt in range(NTN):
                            nc.tensor.matmul(
                                pss[mi][nt][:],
                                xT[f"f8_{j}"][:, :, off : off + P],
                                w8t[j][:, :, nt * N_TILE : (nt + 1) * N_TILE],
                                start=False,
                                stop=(j == JF8 - 1),
                                perf_mode=mybir.MatmulPerfMode.DoubleRow,
                            )
                for mi in range(cm):
                    mt = CHUNK_STARTS[c] + mi
                    y_sb = yout.tile([P, N], BF16, tag="y_sb", name="y_sb")
                    for nt in range(NTN):
                        nc.vector.tensor_copy(
                            y_sb[:, nt * N_TILE : (nt + 1) * N_TILE], pss[mi][nt][:]
                        )
                    nc.scalar.dma_start(y_tiled[mt], y_sb[:])

            # emission: unit transposes interleave with compute chunks so
            # each unit's xbar work lands well before its consuming chunk.
            units = {0: transpose_unit(0), 1: transpose_unit(1)}
            next_u = 2
            for c in range(CHUNKS):
                if c <= 1:
                    matmul_chunk_kc_outer(c, units)
                else:
                    for mi in range(CHUNK_MTS[c]):
                        matmul_mtile(c, mi, units)
                # after every second compute chunk, issue the next unit
                if next_u < UNITS and c % 2 == 1:
                    units[next_u] = transpose_unit(next_u)
                    next_u += 1

    nc.compile()
    return nc


def unpack_w(packed_weight: np.ndarray, weight_scale: np.ndarray):
    planes = [((packed_weight >> (2 * i)) & 3) for i in range(4)]
    w = np.concatenate(planes, axis=0).astype(np.float32) - 1.0  # [N, K]
    ws = np.float32(weight_scale.reshape(-1)[0])
    wTf = np.ascontiguousarray((w / ws).T)  # [K, N] f32
    wT = wTf.astype(ml_dtypes.bfloat16)
    # fp8 pair planes: w8[j, p, i, n] = wT[128*(2j+i) + p, n]
    w8 = np.ascontiguousarray(
        wTf[: KF8 * P].reshape(JF8, 2, P, N).transpose(0, 2, 1, 3)
    ).astype(ml_dtypes.float8_e4m3fn)
    return wT, w8


_CACHE = {}


def run(x: np.ndarray, packed_weight: np.ndarray, weight_scale: np.ndarray,
        trace: bool = False, replays: int = 1, tmpdir=None):
    """x: [B, S, K] bf16 -> y [B, S, N] bf16 (full, unsharded)."""
    key = (replays,)
    if key not in _CACHE:
        _CACHE[key] = build_kernel(replays)
    nc = _CACHE[key]

    B, S, D = x.shape
    M = B * S
    assert M == M_LOC * N_CORES and D == K
    wT, w8 = unpack_w(packed_weight, weight_scale)
    shards = np.ascontiguousarray(np.asarray(x).reshape(N_CORES, M_LOC, K))
    in_maps = [{"x": shards[i], "wT": wT, "w8": w8} for i in range(N_CORES)]
    res = bass_utils.run_bass_kernel_spmd(
        nc, in_maps, core_ids=list(range(N_CORES)), trace=trace, tmpdir=tmpdir
    )
    y = np.stack([res.results[i]["y"] for i in range(N_CORES)], axis=0)
    return y.reshape(B, S, N), res


def kernel(x, packed_weight, weight_scale):
    """Harness entrypoint: FULL inputs -> FULL output.

    x: [4, 8192, 2048] bf16; packed_weight: [512, 2048] uint8;
    weight_scale: [1] bf16.  Returns [4, 8192, 2048] bf16.
    Sharding: data-parallel over tokens across the 8 NeuronCores;
    the (host-unpacked) ternary weight is replicated.
    """
    x = np.asarray(x)
    packed_weight = np.asarray(packed_weight)
    weight_scale = np.asarray(weight_scale)
    y, _ = run(x, packed_weight, weight_scale)
    return y


# revision 43
# speedup vs baseline: 1.3351x; 1.0359x over previous
"""BitLinear158 Trainium2 kernel (per-core body + host driver).

v7: no activation quantization (the reference's per-token int8 scale
cancels algebraically, y ~= x @ w.T within its own quant noise ~0.9%),
plus fp8 DoubleRow matmul on the first 4 of 16 k-chunks. The fp8 part
casts x -> e4m3 on-device (DVE) and runs 256-deep contractions at the
same per-instruction cost as 128-deep bf16 (measured), cutting PE time
12.5%. Total rel err ~1.6e-2 vs the 2e-2 gate (deterministic).

Per core: x_shard [M_LOC, K] bf16 -> y [M_LOC, N] bf16 against
host-unpacked ternary wT [K, N] (bf16 for kc>=4, e4m3 pairs for kc<4).

Queue discipline (HWDGE completion sems assume in-order completion per
queue; xbar transposes complete out of order w.r.t. direct DMAs):
  sync   : DMA transposes only (bf16 xT tiles + uint16 views of x8 pairs)
  scalar : w slices + y stores (direct DMAs only)
  gpsimd : SWDGE - w slices + x slice loads / x8 stores for the fp8 path
  DVE    : bf16->e4m3 casts + PSUM->bf16 drains
"""

import sys

sys.path.insert(0, "/opt/trn_rl_repo")

from contextlib import ExitStack

import numpy as np
import ml_dtypes

import concourse.bass as bass
import concourse.tile as tile
from concourse import bacc, mybir
from concourse import bass_utils

P = 128
M_LOC = 4096      # tokens per core
K = 2048          # in features
N = 2048          # out features
KC = K // P       # 16 k-chunks
KF8 = 6           # k-chunks 0..KF8-1 run in fp8 DoubleRow
JF8 = KF8 // 2    # 2 DoubleRow pair-blocks
NT = M_LOC // P   # 32 m-tiles per core
CHUNK_MTS = [2, 2, 4, 4, 4, 4, 4, 4, 2, 2]
assert sum(CHUNK_MTS) == NT
CHUNK_STARTS = [sum(CHUNK_MTS[:i]) for i in range(len(CHUNK_MTS))]
CHUNKS = len(CHUNK_MTS)
MAX_CHUNK_MT = max(CHUNK_MTS)
N_TILE = 512
NTN = N // N_TILE                  # 4
N_CORES = 8

BF16 = mybir.dt.bfloat16
F32 = mybir.dt.float32
F8 = mybir.dt.float8e4
U16 = mybir.dt.uint16


def build_kernel(replays: int = 1):
    nc = bacc.Bacc("TRN2", target_bir_lowering=False, debug=False, num_devices=N_CORES)
    x = nc.dram_tensor("x", [M_LOC, K], BF16, kind="ExternalInput").ap()
    wT = nc.dram_tensor("wT", [K, N], BF16, kind="ExternalInput").ap()
    w8 = nc.dram_tensor("w8", [JF8, P, 2, N], F8, kind="ExternalInput").ap()
    y = nc.dram_tensor("y", [M_LOC, N], BF16, kind="ExternalOutput").ap()

    y_tiled = y.rearrange("(t p) n -> t p n", p=P)
    # bf16 w pair-blocks j cover kc {2j, 2j+1}; only j >= JF8 are used.
    wT_pair = wT.rearrange("(j two p) n -> j p two n", two=2, p=P)

    with tile.TileContext(nc) as tc, ExitStack() as ctx:
        wbuf = ctx.enter_context(tc.tile_pool(name="wbuf", bufs=1))
        xT_pool = ctx.enter_context(tc.tile_pool(name="xT", bufs=2))
        x8T_pool = ctx.enter_context(tc.tile_pool(name="x8T", bufs=2))
        x8in = ctx.enter_context(tc.tile_pool(name="x8in", bufs=4))
        x8q = ctx.enter_context(tc.tile_pool(name="x8q", bufs=4))
        yout = ctx.enter_context(tc.tile_pool(name="yout", bufs=8))
        psum = ctx.enter_context(tc.tile_pool(name="psum", bufs=8, space="PSUM"))
        dram = ctx.enter_context(tc.tile_pool(name="dram", bufs=1, space="DRAM"))

        # w in PE consumption order, alternating the scalar HWDGE and
        # gpsimd SWDGE queues: bf16 pair slices (kc 4..15) first, fp8
        # pairs last (consumed at the tail of each accumulation group).
        wt = {}
        w8t = [wbuf.tile([P, 2, N], F8, tag=f"w8_{j}", name=f"w8_{j}") for j in range(JF8)]
        ch = [nc.scalar, nc.gpsimd]
        for t, j in enumerate(range(JF8, KC // 2)):
            wt[j] = wbuf.tile([P, 2, N], BF16, tag=f"w{j}", name=f"w{j}")
            ch[t % 2].dma_start(wt[j][:], wT_pair[j])
        for j in range(JF8):
            ch[j % 2].dma_start(w8t[j][:], w8[j])

        def w_ap(kc, nt):
            return wt[kc // 2][:, kc % 2, nt * N_TILE : (nt + 1) * N_TILE]

        for rep in range(replays):
            # Transposes come in units of U_MT m-tiles, decoupled from the
            # (smaller) compute chunks: DMA_TRANSPOSE has a fixed ~1.26us
            # issue cost on the sync queue regardless of size, so fewer,
            # bigger transposes keep the queue far ahead of the PE.
            U_MT = 8
            UNITS = NT // U_MT

            def transpose_unit(u):
                m0 = u * U_MT * P
                tiles = {}
                for kc in range(KC):
                    tt = xT_pool.tile(
                        [P, U_MT * P], BF16, tag=f"xT{kc}", name=f"xT{kc}"
                    )
                    nc.sync.dma_start_transpose(
                        tt[:], x[m0 : m0 + U_MT * P, kc * P : (kc + 1) * P]
                    )
                    tiles[kc] = tt
                # e4m3 conversion after the transpose, entirely in SBUF:
                # DVE casts the transposed bf16 kc<KF8 tiles into [p, 2, m]
                # pair blocks for the DoubleRow matmuls.
                for j in range(JF8):
                    ft = x8T_pool.tile(
                        [P, 2, U_MT * P], F8, tag=f"x8T{j}", name=f"x8T{j}"
                    )
                    for i in range(2):
                        nc.vector.tensor_copy(ft[:, i, :], tiles[2 * j + i][:])
                    tiles[f"f8_{j}"] = ft
                return tiles

            def emit_matmuls(ps, xT, off, nt):
                for kc in range(KF8, KC):
                    nc.tensor.matmul(
                        ps[:],
                        xT[kc][:, off : off + P],
                        w_ap(kc, nt),
                        start=(kc == KF8),
                        stop=(JF8 == 0 and kc == KC - 1),
                    )
                for j in range(JF8):
                    nc.tensor.matmul(
                        ps[:],
                        xT[f"f8_{j}"][:, :, off : off + P],
                        w8t[j][:, :, nt * N_TILE : (nt + 1) * N_TILE],
                        start=False,
                        stop=(j == JF8 - 1),
                        perf_mode=mybir.MatmulPerfMode.DoubleRow,
                    )

            def matmul_mtile(c, mi, units):
                mt = CHUNK_STARTS[c] + mi
                xT = units[mt // U_MT]
                off = (mt % U_MT) * P
                y_sb = yout.tile([P, N], BF16, tag="y_sb", name="y_sb")
                for nt in range(NTN):
                    ps = psum.tile([P, N_TILE], F32, tag="ps", name="ps")
                    emit_matmuls(ps, xT, off, nt)
                    nc.vector.tensor_copy(
                        y_sb[:, nt * N_TILE : (nt + 1) * N_TILE], ps[:]
                    )
                nc.scalar.dma_start(y_tiled[mt], y_sb[:])

            def matmul_chunk_kc_outer(c, units):
                # kc-outer over the whole (small) chunk: the PE consumes each
                # w k-slice as it lands instead of stalling the first m-tile
                # on the full w load. Needs cm*NTN <= 8 PSUM banks.
                xT = units[CHUNK_STARTS[c] // U_MT]
                cm = CHUNK_MTS[c]
                assert cm * NTN <= 8
                pss = [
                    [psum.tile([P, N_TILE], F32, tag="ps", name="ps") for _ in range(NTN)]
                    for _ in range(cm)
                ]
                for kc in range(KF8, KC):
                    for mi in range(cm):
                        off = (CHUNK_STARTS[c] + mi) % U_MT * P
                        for nt in range(NTN):
                            nc.tensor.matmul(
                                pss[mi][nt][:],
                                xT[kc][:, off : off + P],
                                w_ap(kc, nt),
                                start=(kc == KF8),
                                stop=(JF8 == 0 and kc == KC - 1),
                            )
                for j in range(JF8):
                    for mi in range(cm):
                        off = (CHUNK_STARTS[c] + mi) % U_MT * P
                        for nt in range(NTN):
                            nc.tensor.matmul(
                                pss[mi][nt][:],
                                xT[f"f8_{j}"][:, :, off : off + P],
                                w8t[j][:, :, nt * N_TILE : (nt + 1) * N_TILE],
                                start=False,
                                stop=(j == JF8 - 1),
                                perf_mode=mybir.MatmulPerfMode.DoubleRow,
                            )
                for mi in range(cm):
                    mt = CHUNK_STARTS[c] + mi
                    y_sb = yout.tile([P, N], BF16, tag="y_sb", name="y_sb")
                    for nt in range(NTN):
                        nc.vector.tensor_copy(
                            y_sb[:, nt * N_TILE : (nt + 1) * N_TILE], pss[mi][nt][:]
                        )
                    nc.scalar.dma_start(y_tiled[mt], y_sb[:])

            # emission: unit transposes interleave with compute chunks so
            # each unit's xbar work lands well before its consuming chunk.
            units = {0: transpose_unit(0), 1: transpose_unit(1)}
            next_u = 2
            for c in range(CHUNKS):
                if c <= 1:
                    matmul_chunk_kc_outer(c, units)
                else:
                    for mi in range(CHUNK_MTS[c]):
                        matmul_mtile(c, mi, units)
                # after every second compute chunk, issue the next unit
                if next_u < UNITS and c % 2 == 1:
                    units[next_u] = transpose_unit(next_u)
                    next_u += 1

    nc.compile()
    return nc


def unpack_w(packed_weight: np.ndarray, weight_scale: np.ndarray):
    planes = [((packed_weight >> (2 * i)) & 3) for i in range(4)]
    w = np.concatenate(planes, axis=0).astype(np.float32) - 1.0  # [N, K]
    ws = np.float32(weight_scale.reshape(-1)[0])
    wTf = np.ascontiguousarray((w / ws).T)  # [K, N] f32
    wT = wTf.astype(ml_dtypes.bfloat16)
    # fp8 pair planes: w8[j, p, i, n] = wT[128*(2j+i) + p, n]
    w8 = np.ascontiguousarray(
        wTf[: KF8 * P].reshape(JF8, 2, P, N).transpose(0, 2, 1, 3)
    ).astype(ml_dtypes.float8_e4m3fn)
    return wT, w8


_CACHE = {}


def run(x: np.ndarray, packed_weight: np.ndarray, weight_scale: np.ndarray,
        trace: bool = False, replays: int = 1, tmpdir=None):
    """x: [B, S, K] bf16 -> y [B, S, N] bf16 (full, unsharded)."""
    key = (replays,)
    if key not in _CACHE:
        _CACHE[key] = build_kernel(replays)
    nc = _CACHE[key]

    B, S, D = x.shape
    M = B * S
    assert M == M_LOC * N_CORES and D == K
    wT, w8 = unpack_w(packed_weight, weight_scale)
    shards = np.ascontiguousarray(np.asarray(x).reshape(N_CORES, M_LOC, K))
    in_maps = [{"x": shards[i], "wT": wT, "w8": w8} for i in range(N_CORES)]
    res = bass_utils.run_bass_kernel_spmd(
        nc, in_maps, core_ids=list(range(N_CORES)), trace=trace, tmpdir=tmpdir
    )
    y = np.stack([res.results[i]["y"] for i in range(N_CORES)], axis=0)
    return y.reshape(B, S, N), res


def kernel(x, packed_weight, weight_scale):
    """Harness entrypoint: FULL inputs -> FULL output.

    x: [4, 8192, 2048] bf16; packed_weight: [512, 2048] uint8;
    weight_scale: [1] bf16.  Returns [4, 8192, 2048] bf16.
    Sharding: data-parallel over tokens across the 8 NeuronCores;
    the (host-unpacked) ternary weight is replicated.
    """
    x = np.asarray(x)
    packed_weight = np.asarray(packed_weight)
    weight_scale = np.asarray(weight_scale)
    y, _ = run(x, packed_weight, weight_scale)
    return y
